# revision 32
# baseline (speedup 1.0000x reference)
"""Trainium2 Bass kernel for nn_MixGNN (TransformerConv + 3x SAGEConv + BN + gated residual).

Strategy (8 NeuronCores, dst-node sharding, v2 = dedup + masks + fp8 DoubleRow):
  - Pad N 10000 -> 10240; core r owns 1280 dst nodes = 10 tiles of 128 (slots
    sorted ascending by work so the SPMD per-slot chunk counts, max over cores,
    stay tight and the pipeline fills fast).
  - DEDUP: per dst tile, gather each DISTINCT source node once (~3.4K vs ~4.2K
    edges) and carry the edge structure in a host-precomputed multiplicity
    mask M[s, n] (fp8, SBUF-resident, shared by the transformer and all three
    SAGE layers). Gather descriptors are the DMA+Pool bottleneck; dedup plus
    one gather instruction per tile (994ns SWDGE fixed cost each) cuts both.
  - Attention is folded: logits[s,n] = x[n] @ (Wq Wk^T/sqrt(d)) @ x[s]^T; bk
    cancels per-dst in softmax, bq = 0. Per tile: kgt = transposed bf16 gather
    of x for score matmuls; exp on Act (logits O(1), global 1/8 scale folded as
    exp bias to stay in fp8 range); w2 = exp * mask on DVE -> fp8.
  - Value/SAGE aggregation uses fp8 DoubleRow matmuls (0.5 cyc/row): vg = fp8
    row gather; pairs of 128-source chunks contract 256 sources per matmul.
    pagg[n,:D]+denominator accumulate in PSUM; mean normalizes then
    post-multiplies by Wv (linearity) together with the x @ Ws skip path.
  - SAGE: mean aggregation is transposed (paggT[d,n] += vg^T @ mask) feeding
    Wl directly; 1/deg of dst applied per-partition AFTER the Wl matmul
    (separate PSUM) and combined with h @ Wr + bias via DVE; gated residual +
    ReLU epilogue. BN gamma (eval) folded into Wl/Wr columns on host.
  - Halo exchange: 3 AllGathers of fp8 h-table shards (0.33MB/core).
Output: fp32 [10000, 256] (bf16 device output, upcast + slot-unpermuted on host).
"""
import os
import sys
import time

import numpy as np

for _p in ("/opt/trn_rl_repo",):
    if _p not in sys.path:
        sys.path.insert(0, _p)

import ml_dtypes  # noqa: E402
import concourse.bacc as bacc  # noqa: E402
import concourse.mybir as mybir  # noqa: E402
import concourse.tile as tile  # noqa: E402
from concourse.bass_utils import run_bass_kernel_spmd  # noqa: E402

P = 128
D = 256
DJ = D // P           # 2 d-chunks of 128
NC = 8                # cores
L = 3                 # SAGE layers
BN_EPS = 1e-5
N_AG = 3              # AllGathers on the critical path (h0, h1, h2)
EXP_BIAS = -2.0794415416798357  # ln(1/8): folded into exp, cancels in softmax

F32 = mybir.dt.float32
BF16 = mybir.dt.bfloat16
FP8 = mybir.dt.float8e4
I16 = mybir.dt.int16
NP_FP8 = ml_dtypes.float8_e4m3

_nc_cache = {}


def _wrap_idx(a):
    """[S*128] int array -> [128, S*8] int16 wrapped gather-index layout."""
    w16 = a.reshape(-1, 16).T.astype(np.int16)   # [16, S*8]
    return np.tile(w16, (8, 1))                  # replicate to 8 Q7 stripes


def build_nc(n_pad, sh, nt, S_list, oma):
    nocc = os.environ.get("KNOCC") == "1"
    ksm = int(os.environ.get("KSM", "16"))
    kvg = int(os.environ.get("KVG", "4"))
    kkg = int(os.environ.get("KKG", "5"))
    kpsc = int(os.environ.get("KPSC", "2"))
    kpagg = int(os.environ.get("KPAGG", "2"))
    kpmm = int(os.environ.get("KPMM", "2"))
    kptr = int(os.environ.get("KPTR", "1"))
    kgrp = int(os.environ.get("KGRP", "4"))   # chunks per exp group (psum bank)
    S_list = tuple(int(s) for s in S_list)
    key = (n_pad, sh, nt, S_list, round(oma, 9), nocc, ksm, kvg, kkg,
           kpsc, kpagg, kpmm, kptr, kgrp)
    if key in _nc_cache:
        return _nc_cache[key]

    SC = sum(S_list)                 # total source chunks across local tiles
    offs = [0]
    for s in S_list:
        offs.append(offs[-1] + s)
    ndev = 1 if nocc else NC
    nc = bacc.Bacc("TRN2", target_bir_lowering=False, debug=False, num_devices=ndev)

    NW = 9  # packed weights: M, Wv, Ws, Wl0, Wr0, Wl1, Wr1, Wl2, Wr2
    NV = 7  # packed vecs: bv+bs, Bx0, Bx0, Bx1, Bx1, Bx2, Bx2

    xt_in = nc.dram_tensor("xt_in", [P, DJ * sh], BF16, kind="ExternalInput")
    wpack_in = nc.dram_tensor("wpack_in", [P, NW * DJ * D], BF16, kind="ExternalInput")
    vpack_in = nc.dram_tensor("vpack_in", [1, NV * D], BF16, kind="ExternalInput")
    idx_in = nc.dram_tensor("idx_in", [P, SC * 8], I16, kind="ExternalInput")
    msk_in = nc.dram_tensor("msk_in", [P, SC, P], FP8, kind="ExternalInput")
    ivd_in = nc.dram_tensor("ivd_in", [P, nt], F32, kind="ExternalInput")
    xtab_in = nc.dram_tensor("xtab_in", [n_pad, D], BF16, kind="ExternalInput")
    xtab8_in = nc.dram_tensor("xtab8_in", [n_pad, D], FP8, kind="ExternalInput")
    out_dram = nc.dram_tensor("out", [sh, D], BF16, kind="ExternalOutput")

    WM, WV, WS = 0, 1, 2
    WL = [3, 5, 7]
    WR = [4, 6, 8]

    with tile.TileContext(nc) as tc:
        with (
            tc.tile_pool(name="cst", bufs=1) as cst,
            tc.tile_pool(name="sb", bufs=1) as sb,
            tc.tile_pool(name="g", bufs=1) as gp,
            tc.tile_pool(name="sm", bufs=ksm) as smp,
            tc.tile_pool(name="ps", bufs=2, space="PSUM") as ps,
            tc.tile_pool(name="dr", bufs=1, space="DRAM") as dr,
        ):
            # ---------------- constants / inputs to SBUF ----------------
            idx_sb = cst.tile([P, SC * 8], I16)
            _ic = S_list[0] * 8  # first tile's indices land first
            nc.sync.dma_start(out=idx_sb[:, :_ic], in_=idx_in[:, :_ic])
            nc.sync.dma_start(out=idx_sb[:, _ic:], in_=idx_in[:, _ic:])
            msk = cst.tile([P, SC, P], FP8)
            _mc = S_list[0]
            nc.sync.dma_start(out=msk[:, :_mc, :], in_=msk_in[:, :_mc, :])
            for _mi in range(3):
                _c0 = _mc + _mi * (SC - _mc) // 3
                _c1 = _mc + (_mi + 1) * (SC - _mc) // 3
                nc.sync.dma_start(out=msk[:, _c0:_c1, :],
                                  in_=msk_in[:, _c0:_c1, :])
            ivd = cst.tile([P, nt], F32)
            nc.sync.dma_start(out=ivd[:], in_=ivd_in[:])
            wp = cst.tile([P, NW * DJ * D], BF16)
            nc.sync.dma_start(out=wp[:], in_=wpack_in[:])
            vp = cst.tile([1, NV * D], BF16)
            nc.sync.dma_start(out=vp[:], in_=vpack_in[:])
            xt = cst.tile([P, DJ * sh], BF16)
            for _xi in range(4):
                _c0 = _xi * (DJ * sh // 4)
                _c1 = (_xi + 1) * (DJ * sh // 4)
                nc.sync.dma_start(out=xt[:, _c0:_c1], in_=xt_in[:, _c0:_c1])

            ones8 = cst.tile([P, 2, 16], FP8)
            nc.vector.memset(ones8[:], 1.0)
            ones_f1 = cst.tile([1, 1], F32)
            nc.vector.memset(ones_f1[:], 1.0)
            ebias = cst.tile([P, 1], F32)
            nc.vector.memset(ebias[:], EXP_BIAS)
            ones_row = cst.tile([1, P], BF16)
            nc.vector.memset(ones_row[:], 1.0)
            # identity for PE transposes: (iota_row == partition_idx)
            iota_i = cst.tile([P, P], mybir.dt.int32)
            nc.gpsimd.iota(iota_i[:], pattern=[[1, P]], base=0, channel_multiplier=0)
            iota_part = cst.tile([P, 1], mybir.dt.int32)
            nc.gpsimd.iota(iota_part[:], pattern=[[1, 1]], base=0, channel_multiplier=1)
            iota_part_f = cst.tile([P, 1], F32)
            nc.vector.tensor_copy(out=iota_part_f[:], in_=iota_part[:])
            iota_f = cst.tile([P, P], F32)
            nc.vector.tensor_copy(out=iota_f[:], in_=iota_i[:])
            ident_b = cst.tile([P, P], BF16)
            nc.vector.tensor_scalar(
                out=ident_b[:], in0=iota_f[:], scalar1=iota_part_f[:, :1], scalar2=None,
                op0=mybir.AluOpType.is_equal,
            )

            def wslice(w, j):
                return wp[:, (w * DJ + j) * D:(w * DJ + j + 1) * D]

            def vslice(k):
                return vp[:, k * D:(k + 1) * D]  # [1, D] single-partition row

            def xtile(j, t):
                return xt[:, j * sh + t * P: j * sh + (t + 1) * P]

            # ---------------- DRAM tables ----------------
            hag_in = [dr.tile([sh, D], FP8, name=f"hag_in_{i}") for i in range(L)]
            h_full = [dr.tile([n_pad, D], FP8, name=f"h_full_{i}",
                              addr_space=("Local" if nocc else "Shared"))
                      for i in range(L)]

            def allgather(in_t, out_t):
                if nocc:
                    pass  # per-tile h_full writes stand in for the AG
                else:
                    nc.gpsimd.collective_compute(
                        "AllGather", mybir.AluOpType.bypass,
                        replica_groups=[list(range(NC))],
                        ins=[in_t[:]], outs=[out_t[:]],
                    )

            # ---------------- stage 0: aT = M^T X_tile^T per tile ----------------
            # aT[j][d, n]: psc[s, n] = sum_d kgt[d, s] * aT[d, n]
            #            = x[n] @ M @ x[s]^T  (logit of source s vs dst n)
            aT = [sb.tile([P, sh], BF16, name=f"aT_{j}") for j in range(DJ)]
            n0 = 0
            while n0 < sh:
                nn = min(512, sh - n0)
                for j in range(DJ):
                    pq = ps.tile([P, 512], F32, name="pq", tag="pmm", bufs=kpmm)
                    for ki in range(DJ):
                        nc.tensor.matmul(
                            pq[:, :nn],
                            lhsT=wslice(WM, ki)[:, j * P:(j + 1) * P],
                            rhs=xt[:, ki * sh + n0: ki * sh + n0 + nn],
                            start=(ki == 0), stop=(ki == DJ - 1),
                        )
                    nc.scalar.copy(out=aT[j][:, n0:n0 + nn], in_=pq[:, :nn])
                n0 += nn

            # shard-resident activations
            h_cur = sb.tile([P, nt * D], BF16)
            h_nxt = sb.tile([P, nt * D], BF16)
            hT_cur = sb.tile([P, DJ * sh], BF16)
            hT_nxt = sb.tile([P, DJ * sh], BF16)

            def agg_pass(layer, h_prev, hT_prev, h_out, hT_out):
                """layer -1: transformer (h_prev/hT_prev unused); 0..L-1: SAGE."""
                li = layer + 1  # h table index this pass WRITES (0 for transformer)
                for t in range(nt):
                    St = S_list[t]
                    o8 = offs[t] * 8
                    nsrc = St * P
                    # split gathers into even-chunk pieces: finer pipeline
                    # stages (chain latency ~ piece, not tile)
                    hs = ((St // 2) + 1) // 2 * 2
                    pieces = [(0, hs), (hs, St)] if hs < St else [(0, St)]

                    vg = gp.tile([P, St, D], FP8, name="vg", tag="vg", bufs=kvg)
                    src8 = xtab8_in if layer < 0 else h_full[layer]
                    for (ca, cb) in pieces:
                        nc.gpsimd.dma_gather(
                            out_ap=vg[:, ca:cb, :], in_ap=src8[:],
                            idxs_ap=idx_sb[:, o8 + ca * 8: o8 + cb * 8],
                            num_idxs=(cb - ca) * P, num_idxs_reg=(cb - ca) * P,
                            elem_size=D, single_packet=False)
                    kgt_pieces = []
                    if layer < 0:
                        for (ca, cb) in pieces:
                            kgp_t = gp.tile([P, DJ, (cb - ca) * P], BF16,
                                            name="kgt", tag="kgt", bufs=kkg)
                            nc.gpsimd.dma_gather(
                                out_ap=kgp_t[:], in_ap=xtab_in[:],
                                idxs_ap=idx_sb[:, o8 + ca * 8: o8 + cb * 8],
                                num_idxs=(cb - ca) * P, num_idxs_reg=(cb - ca) * P,
                                elem_size=D, transpose=True, single_packet=False)
                            kgt_pieces.append((ca, cb, kgp_t))

                    # pz: bias + dense root term
                    pz = ps.tile([P, D], F32, name="pz", tag="pmm", bufs=kpmm)
                    if layer < 0:
                        nc.tensor.matmul(pz[:], lhsT=ones_row[:], rhs=vslice(0),
                                         start=True, stop=False)
                        for j in range(DJ):
                            nc.tensor.matmul(pz[:], lhsT=xtile(j, t),
                                             rhs=wslice(WS, j),
                                             start=False, stop=False)
                    else:
                        nc.tensor.matmul(pz[:], lhsT=ones_row[:],
                                         rhs=vslice(2 + 2 * layer),
                                         start=True, stop=False)
                        for j in range(DJ):
                            nc.tensor.matmul(
                                pz[:],
                                lhsT=hT_prev[:, j * sh + t * P: j * sh + (t + 1) * P],
                                rhs=wslice(WR[layer], j),
                                start=False, stop=False)

                    if layer < 0:
                        # ---- transformer: scores -> exp -> w2 -> DR agg ----
                        pagg = ps.tile([P, D], F32, name="pagg", tag="pagg",
                                       bufs=kpagg)
                        pden = ps.tile([1, P], F32, name="pden", tag="pden",
                                       bufs=kpagg)
                        npair = St // 2
                        bounds = [b for (a, b) in pieces]
                        cp = 0
                        while cp < St:
                            cb_lim = next(b for b in bounds if b > cp)
                            ng = min(kgrp, cb_lim - cp)
                            psc = ps.tile([P, ng * P], F32, name="psc",
                                          tag="psc", bufs=kpsc)
                            for ci in range(ng):
                                c = cp + ci
                                kge = next(p for p in kgt_pieces
                                           if p[0] <= c < p[1])
                                cof = c - kge[0]
                                for j in range(DJ):
                                    nc.tensor.matmul(
                                        psc[:, ci * P:(ci + 1) * P],
                                        lhsT=kge[2][:, j, cof * P:(cof + 1) * P],
                                        rhs=aT[j][:, t * P:(t + 1) * P],
                                        start=(j == 0), stop=(j == DJ - 1))
                            exps = smp.tile([P, ng * P], BF16, name="exps",
                                            tag="exps", bufs=10)
                            nc.scalar.activation(exps[:], psc[:],
                                                 mybir.ActivationFunctionType.Exp,
                                                 bias=ebias[:, :1])
                            # one mask-mult per group; DR matmuls slice pairs
                            w2 = smp.tile([P, ng, P], FP8, name="w2", tag="w2",
                                          bufs=10)
                            nc.vector.scalar_tensor_tensor(
                                out=w2[:], in0=exps[:],
                                scalar=1.0,
                                in1=msk[:, offs[t] + cp: offs[t] + cp + ng, :],
                                op0=mybir.AluOpType.mult,
                                op1=mybir.AluOpType.mult)
                            for pi in range(ng // 2):
                                p0 = cp + 2 * pi
                                pr = p0 // 2
                                nc.tensor.matmul(
                                    pagg[:], lhsT=w2[:, 2 * pi:2 * pi + 2, :],
                                    rhs=vg[:, p0:p0 + 2, :],
                                    start=(pr == 0), stop=(pr == npair - 1),
                                    perf_mode=mybir.MatmulPerfMode.DoubleRow)
                                nc.tensor.matmul(
                                    pden[:], lhsT=ones8[:, :, 0:1],
                                    rhs=w2[:, 2 * pi:2 * pi + 2, :],
                                    start=(pr == 0), stop=(pr == npair - 1),
                                    perf_mode=mybir.MatmulPerfMode.DoubleRow)
                            cp += ng

                        # ---- epilogue: normalize, Wv + skip, relu ----
                        # denom row [1,128] -> per-partition column via K=1 mm
                        drow = smp.tile([1, P], F32, name="drow")
                        nc.scalar.copy(out=drow[:], in_=pden[:])
                        pdT = ps.tile([P, 1], F32, name="pdT", tag="psc",
                                      bufs=kpsc)
                        nc.tensor.matmul(pdT[:], lhsT=drow[:], rhs=ones_f1[:],
                                         start=True, stop=True)
                        smax = smp.tile([P, 1], F32, name="smax")
                        nc.vector.tensor_scalar(
                            out=smax[:], in0=pdT[:], scalar1=1e-30,
                            scalar2=None, op0=mybir.AluOpType.max)
                        rs = smp.tile([P, 1], F32, name="rs")
                        nc.vector.reciprocal(rs[:], smax[:])
                        mean_x = smp.tile([P, D], BF16, name="mean_x", tag="t1")
                        nc.scalar.activation(mean_x[:], pagg[:],
                                             mybir.ActivationFunctionType.Copy,
                                             scale=rs[:, :1])
                        for j in range(DJ):
                            ptr = ps.tile([P, P], BF16, name="ptr", tag="psc",
                                          bufs=kpsc)
                            nc.tensor.transpose(out=ptr[:],
                                                in_=mean_x[:, j * P:(j + 1) * P],
                                                identity=ident_b[:])
                            mT = smp.tile([P, P], BF16, name="mT", tag="mT")
                            nc.scalar.copy(out=mT[:], in_=ptr[:])
                            nc.tensor.matmul(pz[:], lhsT=mT[:],
                                             rhs=wslice(WV, j),
                                             start=False, stop=(j == DJ - 1))
                        nc.scalar.activation(h_out[:, t * D:(t + 1) * D], pz[:],
                                             mybir.ActivationFunctionType.Relu)
                    else:
                        # ---- SAGE: DR transposed aggregation + Wl + invdeg ----
                        # j-streams on separate tags so consecutive tiles overlap
                        paggT = [ps.tile([P, P], F32, name=f"paggT{j}",
                                         tag=("pagg" if j == 0 else "pden"),
                                         bufs=kpagg) for j in range(DJ)]
                        npair = St // 2
                        for pr in range(npair):
                            p0 = 2 * pr
                            for j in range(DJ):
                                nc.tensor.matmul(
                                    paggT[j][:],
                                    lhsT=vg[:, p0:p0 + 2, j * P:(j + 1) * P],
                                    rhs=msk[:, offs[t] + p0: offs[t] + p0 + 2, :],
                                    start=(pr == 0), stop=(pr == npair - 1),
                                    perf_mode=mybir.MatmulPerfMode.DoubleRow)
                        pz2 = ps.tile([P, D], F32, name="pz2", tag="psc",
                                      bufs=kpsc)
                        for j in range(DJ):
                            mT = smp.tile([P, P], BF16, name="mT", tag="mT")
                            nc.scalar.copy(out=mT[:], in_=paggT[j][:])
                            nc.tensor.matmul(pz2[:], lhsT=mT[:],
                                             rhs=wslice(WL[layer], j),
                                             start=(j == 0), stop=(j == DJ - 1))
                        # invdeg (per dst node) folds into the PSUM->SBUF copy
                        pz2s = smp.tile([P, D], F32, name="pz2s", tag="t4")
                        nc.scalar.activation(pz2s[:], pz2[:],
                                             mybir.ActivationFunctionType.Copy,
                                             scale=ivd[:, t:t + 1])
                        t4 = smp.tile([P, D], F32, name="t4s", tag="t4")
                        nc.vector.scalar_tensor_tensor(
                            out=t4[:], in0=pz2s[:], scalar=1.0,
                            in1=pz[:], op0=mybir.AluOpType.mult,
                            op1=mybir.AluOpType.add)
                        t3 = smp.tile([P, D], F32, name="t3s", tag="t4")
                        nc.vector.scalar_tensor_tensor(
                            out=t3[:], in0=h_prev[:, t * D:(t + 1) * D], scalar=oma,
                            in1=t4[:], op0=mybir.AluOpType.mult,
                            op1=mybir.AluOpType.add)
                        if layer < L - 1:
                            nc.scalar.activation(h_out[:, t * D:(t + 1) * D], t3[:],
                                                 mybir.ActivationFunctionType.Relu)
                        else:
                            hfin = smp.tile([P, D], BF16, name="hfin", tag="t1")
                            nc.scalar.activation(hfin[:], t3[:],
                                                 mybir.ActivationFunctionType.Relu)

                    if layer < L - 1:
                        # fp8 copy of the new h tile for the gather table
                        h8 = smp.tile([P, D], FP8, name="h8", tag="h8")
                        nc.vector.tensor_copy(out=h8[:],
                                              in_=h_out[:, t * D:(t + 1) * D])
                        if nocc:
                            nc.sync.dma_start(out=h_full[li][t * P:(t + 1) * P, :],
                                              in_=h8[:])
                        else:
                            nc.sync.dma_start(out=hag_in[li][t * P:(t + 1) * P, :],
                                              in_=h8[:])
                        for j in range(DJ):
                            ptr2 = ps.tile([P, P], BF16, name="ptr2", tag="psc",
                                           bufs=kpsc)
                            nc.tensor.transpose(
                                out=ptr2[:],
                                in_=h_out[:, t * D + j * P: t * D + (j + 1) * P],
                                identity=ident_b[:])
                            nc.scalar.copy(
                                out=hT_out[:, j * sh + t * P: j * sh + (t + 1) * P],
                                in_=ptr2[:])
                    else:
                        nc.sync.dma_start(out=out_dram[t * P:(t + 1) * P, :],
                                          in_=hfin[:])

                if layer < L - 1:
                    allgather(hag_in[li], h_full[li])

            agg_pass(-1, None, None, h_cur, hT_cur)
            bufs = [(h_cur, hT_cur), (h_nxt, hT_nxt)]
            for i in range(L):
                h_prev, hT_prev = bufs[i % 2]
                h_out, hT_out = bufs[(i + 1) % 2]
                agg_pass(i, h_prev, hT_prev, h_out, hT_out)

    nc.compile()
    _nc_cache[key] = nc
    return nc


def _host_prep(x, src, dst, Wq, bq, Wk, bk, Wv, bv, Ws, bs, Wl, bl, Wr,
               gamma, beta, alpha_res):
    n, d = x.shape
    n_pad = ((n + NC * P - 1) // (NC * P)) * (NC * P)
    sh = n_pad // NC
    nt = sh // P
    n_tiles = n_pad // P

    order = np.argsort(dst, kind="stable")
    src_s, dst_s = src[order], dst[order]
    tile_of = dst_s // P
    counts = np.bincount(tile_of, minlength=n_tiles)
    starts = np.concatenate([[0], np.cumsum(counts)])

    # Per-tile distinct sources (dedup) + multiplicity masks.
    uniq_per_tile = []
    for g in range(n_tiles):
        e0, e1 = starts[g], starts[g + 1]
        uniq = np.unique(src_s[e0:e1])
        uniq_per_tile.append(uniq)
    S_g = np.array([max(1, -(-len(u) // P)) for u in uniq_per_tile])

    # Per-core slot assignment: sort each core's local tiles by distinct-source
    # chunk count DESCENDING (smallest tile last -> short exposed tail at each
    # pass boundary); slot k's static chunk count is the max over cores,
    # rounded up to even for DoubleRow pairing.
    perms = []   # perms[r][k] = local tile index of core r in slot k
    s_sorted = np.empty((NC, nt), np.int64)
    for r in range(NC):
        c_r = S_g[r * nt:(r + 1) * nt]
        p_r = np.argsort(-c_r, kind="stable")
        perms.append(p_r)
        s_sorted[r] = c_r[p_r]
    S_list = s_sorted.max(axis=0)
    S_list = S_list + (S_list % 2)           # even for DR pairs
    S_list = np.maximum(S_list, 2).astype(np.int64)
    SC = int(S_list.sum())
    offs = np.concatenate([[0], np.cumsum(S_list)]).astype(np.int64)

    # Slot-ordered DRAM node tables: position (r*nt + k)*P + p holds node
    # (r*nt + perms[r][k])*P + p; gather indices address positions.
    invperms = [np.argsort(p) for p in perms]
    pos_of_tile = np.empty(n_tiles, np.int64)
    for r in range(NC):
        pos_of_tile[r * nt:(r + 1) * nt] = r * nt + invperms[r]
    ar = np.arange(n_pad)
    pos_of_node = pos_of_tile[ar // P] * P + (ar % P)

    deg = np.bincount(dst, minlength=n_pad).astype(np.float32)
    invdeg_full = 1.0 / np.maximum(deg, 1.0)

    al = 1.0 / (1.0 + np.exp(-alpha_res))
    oma = float(1.0 - al)
    bn_scale = 1.0 / np.sqrt(1.0 + BN_EPS)
    scale = 1.0 / np.sqrt(float(d))

    x_pad = np.zeros((n_pad, D), np.float32)
    x_pad[:n] = x
    xT = x_pad.T.copy()
    xtab = np.zeros((n_pad, D), ml_dtypes.bfloat16)
    xtab[pos_of_node] = x_pad.astype(ml_dtypes.bfloat16)
    xtab8 = np.zeros((n_pad, D), NP_FP8)
    xtab8[pos_of_node] = x_pad.astype(NP_FP8)

    # fold attention: logits = scale * (x[dst] @ Wq + bq) . (x[src] @ Wk + bk)
    #   = x[dst] @ M @ x[src]^T  (+ per-dst const, cancels in softmax; bq = 0)
    M = (Wq @ Wk.T) * scale
    Gx = [al * bn_scale * gamma[i] for i in range(L)]
    Wlg = [Wl[i] * Gx[i][None, :] for i in range(L)]
    Wrg = [Wr[i] * Gx[i][None, :] for i in range(L)]
    weights = [M, Wv, Ws, Wlg[0], Wrg[0], Wlg[1], Wrg[1], Wlg[2], Wrg[2]]
    NW = len(weights)
    wpack = np.empty((P, NW * DJ * D), np.float32)
    for w, W in enumerate(weights):
        for j in range(DJ):
            wpack[:, (w * DJ + j) * D:(w * DJ + j + 1) * D] = W[j * P:(j + 1) * P, :]
    wpack = wpack.astype(ml_dtypes.bfloat16)

    Bx = [al * (bl[i] * bn_scale * gamma[i] + beta[i]) for i in range(L)]
    vecs = [bv + bs, Bx[0], Bx[0], Bx[1], Bx[1], Bx[2], Bx[2]]
    vpack = np.concatenate(vecs)[None, :].astype(ml_dtypes.bfloat16)

    in_maps = []
    for r in range(NC):
        idx_arr = np.zeros((P, SC * 8), np.int16)
        msk_arr = np.zeros((P, SC, P), NP_FP8)
        ivd_arr = np.zeros((P, nt), np.float32)
        for k in range(nt):
            tloc = int(perms[r][k])
            St = int(S_list[k])
            g = r * nt + tloc
            uniq = uniq_per_tile[g]
            nu = len(uniq)
            e0, e1 = starts[g], starts[g + 1]
            # multiplicity mask [St*P src-slot, P dst-local]
            mask = np.zeros((St * P, P), np.float32)
            if nu:
                inv = np.searchsorted(uniq, src_s[e0:e1])
                np.add.at(mask, (inv, dst_s[e0:e1] - g * P), 1.0)
            srcs = np.zeros(St * P, np.int64)
            srcs[:nu] = pos_of_node[uniq]
            o = int(offs[k])
            idx_arr[:, o * 8:(o + St) * 8] = _wrap_idx(srcs)
            # SBUF layout [128 part=src%128, chunk, dstcol]
            m3 = mask.reshape(St, P, P).transpose(1, 0, 2)
            msk_arr[:, o:o + St, :] = m3.astype(NP_FP8)
            ivd_arr[:, k] = invdeg_full[g * P:(g + 1) * P]
        pr = perms[r]

        xt_r = np.empty((P, DJ * sh), np.float32)
        for j in range(DJ):
            xs = xT[j * P:(j + 1) * P, r * sh:(r + 1) * sh]      # [P, sh]
            xs = xs.reshape(P, nt, P)[:, pr, :].reshape(P, sh)   # permute tiles
            xt_r[:, j * sh:(j + 1) * sh] = xs

        in_maps.append({
            "xt_in": xt_r.astype(ml_dtypes.bfloat16),
            "wpack_in": wpack,
            "vpack_in": vpack,
            "idx_in": idx_arr,
            "msk_in": msk_arr,
            "ivd_in": ivd_arr,
            "xtab_in": xtab,
            "xtab8_in": xtab8,
        })
    return in_maps, perms, (n_pad, sh, nt, tuple(int(s) for s in S_list), oma)


def kernel(**inputs):
    x = np.asarray(inputs["x"], np.float32)
    edge_index = np.asarray(inputs["edge_index"])
    args = dict(
        Wq=np.asarray(inputs["Wq"], np.float32), bq=np.asarray(inputs["bq"], np.float32),
        Wk=np.asarray(inputs["Wk"], np.float32), bk=np.asarray(inputs["bk"], np.float32),
        Wv=np.asarray(inputs["Wv"], np.float32), bv=np.asarray(inputs["bv"], np.float32),
        Ws=np.asarray(inputs["Ws"], np.float32), bs=np.asarray(inputs["bs"], np.float32),
        Wl=np.asarray(inputs["Wl"], np.float32), bl=np.asarray(inputs["bl"], np.float32),
        Wr=np.asarray(inputs["Wr"], np.float32),
        gamma=np.asarray(inputs["gamma"], np.float32),
        beta=np.asarray(inputs["beta"], np.float32),
        alpha_res=float(np.asarray(inputs["alpha_res"])),
    )
    src = edge_index[0].astype(np.int64)
    dst = edge_index[1].astype(np.int64)

    in_maps, perms, params = _host_prep(x, src, dst, **args)
    n_pad, sh, nt, S_list, oma = params
    t0 = time.time()
    nc = build_nc(*params)
    print(f"[kernel] build+compile {time.time()-t0:.1f}s", flush=True)
    t0 = time.time()
    res = run_bass_kernel_spmd(nc, in_maps, core_ids=list(range(NC)))
    print(f"[kernel] run {time.time()-t0:.1f}s", flush=True)
    # rows come back slot-ordered; un-permute to natural node order
    outs = []
    for r in range(NC):
        o = np.asarray(res.results[r]["out"]).astype(np.float32).reshape(nt, P, D)
        outs.append(o[np.argsort(perms[r])].reshape(sh, D))
    out = np.concatenate(outs, axis=0)
    return out[:x.shape[0]]


# revision 38
# speedup vs baseline: 1.0483x; 1.0483x over previous
"""Trainium2 Bass kernel for nn_MixGNN (TransformerConv + 3x SAGEConv + BN + gated residual).

Strategy (8 NeuronCores, dst-node sharding, v2 = dedup + masks + fp8 DoubleRow):
  - Pad N 10000 -> 10240; core r owns 1280 dst nodes = 10 tiles of 128 (slots
    sorted ascending by work so the SPMD per-slot chunk counts, max over cores,
    stay tight and the pipeline fills fast).
  - DEDUP: per dst tile, gather each DISTINCT source node once (~3.4K vs ~4.2K
    edges) and carry the edge structure in a host-precomputed multiplicity
    mask M[s, n] (fp8, SBUF-resident, shared by the transformer and all three
    SAGE layers). Gather descriptors are the DMA+Pool bottleneck; dedup plus
    one gather instruction per tile (994ns SWDGE fixed cost each) cuts both.
  - Attention is folded: logits[s,n] = x[n] @ (Wq Wk^T/sqrt(d)) @ x[s]^T; bk
    cancels per-dst in softmax, bq = 0. Per tile: kgt = transposed bf16 gather
    of x for score matmuls; exp on Act (logits O(1), global 1/8 scale folded as
    exp bias to stay in fp8 range); w2 = exp * mask on DVE -> fp8.
  - Value/SAGE aggregation uses fp8 DoubleRow matmuls (0.5 cyc/row): vg = fp8
    row gather; pairs of 128-source chunks contract 256 sources per matmul.
    pagg[n,:D]+denominator accumulate in PSUM; mean normalizes then
    post-multiplies by Wv (linearity) together with the x @ Ws skip path.
  - SAGE: mean aggregation is transposed (paggT[d,n] += vg^T @ mask) feeding
    Wl directly; 1/deg of dst applied per-partition AFTER the Wl matmul
    (separate PSUM) and combined with h @ Wr + bias via DVE; gated residual +
    ReLU epilogue. BN gamma (eval) folded into Wl/Wr columns on host.
  - Halo exchange: 3 AllGathers of fp8 h-table shards (0.33MB/core).
Output: fp32 [10000, 256] (bf16 device output, upcast + slot-unpermuted on host).
"""
import os
import sys
import time

import numpy as np

for _p in ("/opt/trn_rl_repo",):
    if _p not in sys.path:
        sys.path.insert(0, _p)

import ml_dtypes  # noqa: E402
import concourse.bacc as bacc  # noqa: E402
import concourse.mybir as mybir  # noqa: E402
import concourse.tile as tile  # noqa: E402
from concourse.bass_utils import run_bass_kernel_spmd  # noqa: E402

P = 128
D = 256
DJ = D // P           # 2 d-chunks of 128
NC = 8                # cores
L = 3                 # SAGE layers
BN_EPS = 1e-5
N_AG = 3              # AllGathers on the critical path (h0, h1, h2)
EXP_BIAS = -2.0794415416798357  # ln(1/8): folded into exp, cancels in softmax

F32 = mybir.dt.float32
BF16 = mybir.dt.bfloat16
FP8 = mybir.dt.float8e4
I16 = mybir.dt.int16
NP_FP8 = ml_dtypes.float8_e4m3

_nc_cache = {}


def _wrap_idx(a):
    """[S*128] int array -> [128, S*8] int16 wrapped gather-index layout."""
    w16 = a.reshape(-1, 16).T.astype(np.int16)   # [16, S*8]
    return np.tile(w16, (8, 1))                  # replicate to 8 Q7 stripes


def build_nc(n_pad, sh, nt, S_list, oma):
    nocc = os.environ.get("KNOCC") == "1"
    ksm = int(os.environ.get("KSM", "16"))
    kvg = int(os.environ.get("KVG", "4"))
    kkg = int(os.environ.get("KKG", "5"))
    kpsc = int(os.environ.get("KPSC", "4"))
    kpagg = int(os.environ.get("KPAGG", "2"))
    kpmm = int(os.environ.get("KPMM", "2"))
    kptr = int(os.environ.get("KPTR", "1"))
    kgrp = int(os.environ.get("KGRP", "4"))   # chunks per exp group (psum bank)
    S_list = tuple(int(s) for s in S_list)
    key = (n_pad, sh, nt, S_list, round(oma, 9), nocc, ksm, kvg, kkg,
           kpsc, kpagg, kpmm, kptr, kgrp)
    if key in _nc_cache:
        return _nc_cache[key]

    SC = sum(S_list)                 # total source chunks across local tiles
    offs = [0]
    for s in S_list:
        offs.append(offs[-1] + s)
    ndev = 1 if nocc else NC
    nc = bacc.Bacc("TRN2", target_bir_lowering=False, debug=False, num_devices=ndev)

    NW = 9  # packed weights: M, Wv, Ws, Wl0, Wr0, Wl1, Wr1, Wl2, Wr2
    NV = 7  # packed vecs: bv+bs, Bx0, Bx0, Bx1, Bx1, Bx2, Bx2

    xt_in = nc.dram_tensor("xt_in", [P, DJ * sh], BF16, kind="ExternalInput")
    wpack_in = nc.dram_tensor("wpack_in", [P, NW * DJ * D], BF16, kind="ExternalInput")
    vpack_in = nc.dram_tensor("vpack_in", [1, NV * D], BF16, kind="ExternalInput")
    idx_in = nc.dram_tensor("idx_in", [P, SC * 8], I16, kind="ExternalInput")
    msk_in = nc.dram_tensor("msk_in", [P, SC, P], FP8, kind="ExternalInput")
    ivd_in = nc.dram_tensor("ivd_in", [P, nt], F32, kind="ExternalInput")
    xtab_in = nc.dram_tensor("xtab_in", [n_pad, D], BF16, kind="ExternalInput")
    xtab8_in = nc.dram_tensor("xtab8_in", [n_pad, D], FP8, kind="ExternalInput")
    out_dram = nc.dram_tensor("out", [sh, D], BF16, kind="ExternalOutput")

    WM, WV, WS = 0, 1, 2
    WL = [3, 5, 7]
    WR = [4, 6, 8]

    with tile.TileContext(nc) as tc:
        with (
            tc.tile_pool(name="cst", bufs=1) as cst,
            tc.tile_pool(name="sb", bufs=1) as sb,
            tc.tile_pool(name="g", bufs=1) as gp,
            tc.tile_pool(name="sm", bufs=ksm) as smp,
            tc.tile_pool(name="ps", bufs=2, space="PSUM") as ps,
            tc.tile_pool(name="dr", bufs=1, space="DRAM") as dr,
        ):
            # ---------------- constants / inputs to SBUF ----------------
            idx_sb = cst.tile([P, SC * 8], I16)
            _ic = S_list[0] * 8  # first tile's indices land first
            nc.sync.dma_start(out=idx_sb[:, :_ic], in_=idx_in[:, :_ic])
            nc.sync.dma_start(out=idx_sb[:, _ic:], in_=idx_in[:, _ic:])
            msk = cst.tile([P, SC, P], FP8)
            _mc = S_list[0]
            nc.sync.dma_start(out=msk[:, :_mc, :], in_=msk_in[:, :_mc, :])
            for _mi in range(3):
                _c0 = _mc + _mi * (SC - _mc) // 3
                _c1 = _mc + (_mi + 1) * (SC - _mc) // 3
                nc.sync.dma_start(out=msk[:, _c0:_c1, :],
                                  in_=msk_in[:, _c0:_c1, :])
            ivd = cst.tile([P, nt], F32)
            nc.sync.dma_start(out=ivd[:], in_=ivd_in[:])
            wp = cst.tile([P, NW * DJ * D], BF16)
            nc.sync.dma_start(out=wp[:], in_=wpack_in[:])
            vp = cst.tile([1, NV * D], BF16)
            nc.sync.dma_start(out=vp[:], in_=vpack_in[:])
            xt = cst.tile([P, DJ * sh], BF16)
            for _xi in range(4):
                _c0 = _xi * (DJ * sh // 4)
                _c1 = (_xi + 1) * (DJ * sh // 4)
                nc.sync.dma_start(out=xt[:, _c0:_c1], in_=xt_in[:, _c0:_c1])

            ones81 = cst.tile([P, 1], FP8)
            nc.vector.memset(ones81[:], 1.0)
            ebias = cst.tile([P, 1], F32)
            nc.vector.memset(ebias[:], EXP_BIAS)
            ones_row = cst.tile([1, P], BF16)
            nc.vector.memset(ones_row[:], 1.0)
            # identity for PE transposes: (iota_row == partition_idx)
            iota_i = cst.tile([P, P], mybir.dt.int32)
            nc.gpsimd.iota(iota_i[:], pattern=[[1, P]], base=0, channel_multiplier=0)
            iota_part = cst.tile([P, 1], mybir.dt.int32)
            nc.gpsimd.iota(iota_part[:], pattern=[[1, 1]], base=0, channel_multiplier=1)
            iota_part_f = cst.tile([P, 1], F32)
            nc.vector.tensor_copy(out=iota_part_f[:], in_=iota_part[:])
            iota_f = cst.tile([P, P], F32)
            nc.vector.tensor_copy(out=iota_f[:], in_=iota_i[:])
            ident_b = cst.tile([P, P], BF16)
            nc.vector.tensor_scalar(
                out=ident_b[:], in0=iota_f[:], scalar1=iota_part_f[:, :1], scalar2=None,
                op0=mybir.AluOpType.is_equal,
            )

            def wslice(w, j):
                return wp[:, (w * DJ + j) * D:(w * DJ + j + 1) * D]

            def vslice(k):
                return vp[:, k * D:(k + 1) * D]  # [1, D] single-partition row

            def xtile(j, t):
                return xt[:, j * sh + t * P: j * sh + (t + 1) * P]

            # ---------------- DRAM tables ----------------
            hag_in = [dr.tile([sh, D], FP8, name=f"hag_in_{i}") for i in range(L)]
            h_full = [dr.tile([n_pad, D], FP8, name=f"h_full_{i}",
                              addr_space=("Local" if nocc else "Shared"))
                      for i in range(L)]

            def allgather(in_t, out_t):
                if nocc:
                    pass  # per-tile h_full writes stand in for the AG
                else:
                    nc.gpsimd.collective_compute(
                        "AllGather", mybir.AluOpType.bypass,
                        replica_groups=[list(range(NC))],
                        ins=[in_t[:]], outs=[out_t[:]],
                    )

            # ---------------- stage 0: aT = M^T X_tile^T per tile ----------------
            # aT[j][d, n]: psc[s, n] = sum_d kgt[d, s] * aT[d, n]
            #            = x[n] @ M @ x[s]^T  (logit of source s vs dst n)
            aT = [sb.tile([P, sh], BF16, name=f"aT_{j}") for j in range(DJ)]
            n0 = 0
            while n0 < sh:
                nn = min(512, sh - n0)
                for j in range(DJ):
                    pq = ps.tile([P, 512], F32, name="pq", tag="pmm", bufs=kpmm)
                    for ki in range(DJ):
                        nc.tensor.matmul(
                            pq[:, :nn],
                            lhsT=wslice(WM, ki)[:, j * P:(j + 1) * P],
                            rhs=xt[:, ki * sh + n0: ki * sh + n0 + nn],
                            start=(ki == 0), stop=(ki == DJ - 1),
                        )
                    nc.scalar.copy(out=aT[j][:, n0:n0 + nn], in_=pq[:, :nn])
                n0 += nn

            # shard-resident activations
            h_cur = sb.tile([P, nt * D], BF16)
            h_nxt = sb.tile([P, nt * D], BF16)
            hT_cur = sb.tile([P, DJ * sh], BF16)
            hT_nxt = sb.tile([P, DJ * sh], BF16)

            def agg_pass(layer, h_prev, hT_prev, h_out, hT_out):
                """layer -1: transformer (h_prev/hT_prev unused); 0..L-1: SAGE."""
                li = layer + 1  # h table index this pass WRITES (0 for transformer)
                for t in range(nt):
                    St = S_list[t]
                    o8 = offs[t] * 8
                    nsrc = St * P
                    # split gathers into even-chunk pieces: finer pipeline
                    # stages (chain latency ~ piece, not tile)
                    hs = ((St // 2) + 1) // 2 * 2
                    pieces = [(0, hs), (hs, St)] if hs < St else [(0, St)]

                    vg = gp.tile([P, St, D], FP8, name="vg", tag="vg", bufs=kvg)
                    src8 = xtab8_in if layer < 0 else h_full[layer]
                    for (ca, cb) in pieces:
                        nc.gpsimd.dma_gather(
                            out_ap=vg[:, ca:cb, :], in_ap=src8[:],
                            idxs_ap=idx_sb[:, o8 + ca * 8: o8 + cb * 8],
                            num_idxs=(cb - ca) * P, num_idxs_reg=(cb - ca) * P,
                            elem_size=D, single_packet=False)
                    kgt_pieces = []
                    if layer < 0:
                        for (ca, cb) in pieces:
                            kgp_t = gp.tile([P, DJ, (cb - ca) * P], BF16,
                                            name="kgt", tag="kgt", bufs=kkg)
                            nc.gpsimd.dma_gather(
                                out_ap=kgp_t[:], in_ap=xtab_in[:],
                                idxs_ap=idx_sb[:, o8 + ca * 8: o8 + cb * 8],
                                num_idxs=(cb - ca) * P, num_idxs_reg=(cb - ca) * P,
                                elem_size=D, transpose=True, single_packet=False)
                            kgt_pieces.append((ca, cb, kgp_t))

                    # pz: bias + dense root term
                    pz = ps.tile([P, D], F32, name="pz", tag="pmm", bufs=kpmm)
                    if layer < 0:
                        nc.tensor.matmul(pz[:], lhsT=ones_row[:], rhs=vslice(0),
                                         start=True, stop=False)
                        for j in range(DJ):
                            nc.tensor.matmul(pz[:], lhsT=xtile(j, t),
                                             rhs=wslice(WS, j),
                                             start=False, stop=False)
                    else:
                        nc.tensor.matmul(pz[:], lhsT=ones_row[:],
                                         rhs=vslice(2 + 2 * layer),
                                         start=True, stop=False)
                        for j in range(DJ):
                            nc.tensor.matmul(
                                pz[:],
                                lhsT=hT_prev[:, j * sh + t * P: j * sh + (t + 1) * P],
                                rhs=wslice(WR[layer], j),
                                start=False, stop=False)

                    if layer < 0:
                        # ---- transformer: scores -> exp -> w2 -> DR agg ----
                        # denominator rides in the same PSUM bank (col D)
                        pagg = ps.tile([P, D + 1], F32, name="pagg", tag="pagg",
                                       bufs=kpagg)
                        npair = St // 2
                        bounds = [b for (a, b) in pieces]
                        cp = 0
                        while cp < St:
                            cb_lim = next(b for b in bounds if b > cp)
                            ng = min(kgrp, cb_lim - cp)
                            psc = ps.tile([P, ng * P], F32, name="psc",
                                          tag="psc", bufs=kpsc)
                            for ci in range(ng):
                                c = cp + ci
                                kge = next(p for p in kgt_pieces
                                           if p[0] <= c < p[1])
                                cof = c - kge[0]
                                for j in range(DJ):
                                    nc.tensor.matmul(
                                        psc[:, ci * P:(ci + 1) * P],
                                        lhsT=kge[2][:, j, cof * P:(cof + 1) * P],
                                        rhs=aT[j][:, t * P:(t + 1) * P],
                                        start=(j == 0), stop=(j == DJ - 1))
                            exps = smp.tile([P, ng * P], BF16, name="exps",
                                            tag="exps", bufs=10)
                            nc.scalar.activation(exps[:], psc[:],
                                                 mybir.ActivationFunctionType.Exp,
                                                 bias=ebias[:, :1])
                            # one mask-mult per group; DR matmuls slice pairs
                            w2 = smp.tile([P, ng, P], FP8, name="w2", tag="w2",
                                          bufs=10)
                            nc.vector.scalar_tensor_tensor(
                                out=w2[:], in0=exps[:],
                                scalar=1.0,
                                in1=msk[:, offs[t] + cp: offs[t] + cp + ng, :],
                                op0=mybir.AluOpType.mult,
                                op1=mybir.AluOpType.mult)
                            for pi in range(ng // 2):
                                p0 = cp + 2 * pi
                                pr = p0 // 2
                                nc.tensor.matmul(
                                    pagg[:, :D], lhsT=w2[:, 2 * pi:2 * pi + 2, :],
                                    rhs=vg[:, p0:p0 + 2, :],
                                    start=(pr == 0), stop=(pr == npair - 1),
                                    perf_mode=mybir.MatmulPerfMode.DoubleRow)
                                for i in range(2):
                                    nc.tensor.matmul(
                                        pagg[:, D:D + 1],
                                        lhsT=w2[:, 2 * pi + i, :],
                                        rhs=ones81[:],
                                        start=False,
                                        stop=(pr == npair - 1 and i == 1))
                            cp += ng

                        # ---- epilogue: normalize, Wv + skip, relu ----
                        smax = smp.tile([P, 1], F32, name="smax")
                        nc.vector.tensor_scalar(
                            out=smax[:], in0=pagg[:, D:D + 1], scalar1=1e-30,
                            scalar2=None, op0=mybir.AluOpType.max)
                        rs = smp.tile([P, 1], F32, name="rs")
                        nc.vector.reciprocal(rs[:], smax[:])
                        mean_x = smp.tile([P, D], BF16, name="mean_x", tag="t1")
                        nc.scalar.activation(mean_x[:], pagg[:, :D],
                                             mybir.ActivationFunctionType.Copy,
                                             scale=rs[:, :1])
                        for j in range(DJ):
                            ptr = ps.tile([P, P], BF16, name="ptr", tag="psc",
                                          bufs=kpsc)
                            nc.tensor.transpose(out=ptr[:],
                                                in_=mean_x[:, j * P:(j + 1) * P],
                                                identity=ident_b[:])
                            mT = smp.tile([P, P], BF16, name="mT", tag="mT")
                            nc.scalar.copy(out=mT[:], in_=ptr[:])
                            nc.tensor.matmul(pz[:], lhsT=mT[:],
                                             rhs=wslice(WV, j),
                                             start=False, stop=(j == DJ - 1))
                        nc.scalar.activation(h_out[:, t * D:(t + 1) * D], pz[:],
                                             mybir.ActivationFunctionType.Relu)
                    else:
                        # ---- SAGE: DR transposed aggregation + Wl + invdeg ----
                        # j-streams on separate tags so consecutive tiles overlap
                        paggT = [ps.tile([P, P], F32, name=f"paggT{j}",
                                         tag=("pagg" if j == 0 else "psc"),
                                         bufs=(kpagg if j == 0 else kpsc))
                                 for j in range(DJ)]
                        npair = St // 2
                        for pr in range(npair):
                            p0 = 2 * pr
                            for j in range(DJ):
                                nc.tensor.matmul(
                                    paggT[j][:],
                                    lhsT=vg[:, p0:p0 + 2, j * P:(j + 1) * P],
                                    rhs=msk[:, offs[t] + p0: offs[t] + p0 + 2, :],
                                    start=(pr == 0), stop=(pr == npair - 1),
                                    perf_mode=mybir.MatmulPerfMode.DoubleRow)
                        pz2 = ps.tile([P, D], F32, name="pz2", tag="psc",
                                      bufs=kpsc)
                        for j in range(DJ):
                            mT = smp.tile([P, P], BF16, name="mT", tag="mT")
                            nc.scalar.copy(out=mT[:], in_=paggT[j][:])
                            nc.tensor.matmul(pz2[:], lhsT=mT[:],
                                             rhs=wslice(WL[layer], j),
                                             start=(j == 0), stop=(j == DJ - 1))
                        # invdeg (per dst node) folds into the PSUM->SBUF copy
                        pz2s = smp.tile([P, D], F32, name="pz2s", tag="t4")
                        nc.scalar.activation(pz2s[:], pz2[:],
                                             mybir.ActivationFunctionType.Copy,
                                             scale=ivd[:, t:t + 1])
                        t4 = smp.tile([P, D], F32, name="t4s", tag="t4")
                        nc.vector.scalar_tensor_tensor(
                            out=t4[:], in0=pz2s[:], scalar=1.0,
                            in1=pz[:], op0=mybir.AluOpType.mult,
                            op1=mybir.AluOpType.add)
                        t3 = smp.tile([P, D], F32, name="t3s", tag="t4")
                        nc.vector.scalar_tensor_tensor(
                            out=t3[:], in0=h_prev[:, t * D:(t + 1) * D], scalar=oma,
                            in1=t4[:], op0=mybir.AluOpType.mult,
                            op1=mybir.AluOpType.add)
                        if layer < L - 1:
                            nc.scalar.activation(h_out[:, t * D:(t + 1) * D], t3[:],
                                                 mybir.ActivationFunctionType.Relu)
                        else:
                            hfin = smp.tile([P, D], BF16, name="hfin", tag="t1")
                            nc.scalar.activation(hfin[:], t3[:],
                                                 mybir.ActivationFunctionType.Relu)

                    if layer < L - 1:
                        # fp8 copy of the new h tile for the gather table
                        h8 = smp.tile([P, D], FP8, name="h8", tag="h8")
                        nc.vector.tensor_copy(out=h8[:],
                                              in_=h_out[:, t * D:(t + 1) * D])
                        if nocc:
                            nc.sync.dma_start(out=h_full[li][t * P:(t + 1) * P, :],
                                              in_=h8[:])
                        else:
                            nc.sync.dma_start(out=hag_in[li][t * P:(t + 1) * P, :],
                                              in_=h8[:])
                        for j in range(DJ):
                            ptr2 = ps.tile([P, P], BF16, name="ptr2", tag="psc",
                                           bufs=kpsc)
                            nc.tensor.transpose(
                                out=ptr2[:],
                                in_=h_out[:, t * D + j * P: t * D + (j + 1) * P],
                                identity=ident_b[:])
                            nc.scalar.copy(
                                out=hT_out[:, j * sh + t * P: j * sh + (t + 1) * P],
                                in_=ptr2[:])
                    else:
                        nc.sync.dma_start(out=out_dram[t * P:(t + 1) * P, :],
                                          in_=hfin[:])

                if layer < L - 1:
                    allgather(hag_in[li], h_full[li])

            agg_pass(-1, None, None, h_cur, hT_cur)
            bufs = [(h_cur, hT_cur), (h_nxt, hT_nxt)]
            for i in range(L):
                h_prev, hT_prev = bufs[i % 2]
                h_out, hT_out = bufs[(i + 1) % 2]
                agg_pass(i, h_prev, hT_prev, h_out, hT_out)

    nc.compile()
    _nc_cache[key] = nc
    return nc


def _host_prep(x, src, dst, Wq, bq, Wk, bk, Wv, bv, Ws, bs, Wl, bl, Wr,
               gamma, beta, alpha_res):
    n, d = x.shape
    n_pad = ((n + NC * P - 1) // (NC * P)) * (NC * P)
    sh = n_pad // NC
    nt = sh // P
    n_tiles = n_pad // P

    order = np.argsort(dst, kind="stable")
    src_s, dst_s = src[order], dst[order]
    tile_of = dst_s // P
    counts = np.bincount(tile_of, minlength=n_tiles)
    starts = np.concatenate([[0], np.cumsum(counts)])

    # Per-tile distinct sources (dedup) + multiplicity masks.
    uniq_per_tile = []
    for g in range(n_tiles):
        e0, e1 = starts[g], starts[g + 1]
        uniq = np.unique(src_s[e0:e1])
        uniq_per_tile.append(uniq)
    S_g = np.array([max(1, -(-len(u) // P)) for u in uniq_per_tile])

    # Per-core slot assignment: sort each core's local tiles by distinct-source
    # chunk count DESCENDING (smallest tile last -> short exposed tail at each
    # pass boundary); slot k's static chunk count is the max over cores,
    # rounded up to even for DoubleRow pairing.
    perms = []   # perms[r][k] = local tile index of core r in slot k
    s_sorted = np.empty((NC, nt), np.int64)
    for r in range(NC):
        c_r = S_g[r * nt:(r + 1) * nt]
        p_r = np.argsort(-c_r, kind="stable")
        perms.append(p_r)
        s_sorted[r] = c_r[p_r]
    S_list = s_sorted.max(axis=0)
    S_list = S_list + (S_list % 2)           # even for DR pairs
    S_list = np.maximum(S_list, 2).astype(np.int64)
    SC = int(S_list.sum())
    offs = np.concatenate([[0], np.cumsum(S_list)]).astype(np.int64)

    # Slot-ordered DRAM node tables: position (r*nt + k)*P + p holds node
    # (r*nt + perms[r][k])*P + p; gather indices address positions.
    invperms = [np.argsort(p) for p in perms]
    pos_of_tile = np.empty(n_tiles, np.int64)
    for r in range(NC):
        pos_of_tile[r * nt:(r + 1) * nt] = r * nt + invperms[r]
    ar = np.arange(n_pad)
    pos_of_node = pos_of_tile[ar // P] * P + (ar % P)

    deg = np.bincount(dst, minlength=n_pad).astype(np.float32)
    invdeg_full = 1.0 / np.maximum(deg, 1.0)

    al = 1.0 / (1.0 + np.exp(-alpha_res))
    oma = float(1.0 - al)
    bn_scale = 1.0 / np.sqrt(1.0 + BN_EPS)
    scale = 1.0 / np.sqrt(float(d))

    x_pad = np.zeros((n_pad, D), np.float32)
    x_pad[:n] = x
    xT = x_pad.T.copy()
    xtab = np.zeros((n_pad, D), ml_dtypes.bfloat16)
    xtab[pos_of_node] = x_pad.astype(ml_dtypes.bfloat16)
    xtab8 = np.zeros((n_pad, D), NP_FP8)
    xtab8[pos_of_node] = x_pad.astype(NP_FP8)

    # fold attention: logits = scale * (x[dst] @ Wq + bq) . (x[src] @ Wk + bk)
    #   = x[dst] @ M @ x[src]^T  (+ per-dst const, cancels in softmax; bq = 0)
    M = (Wq @ Wk.T) * scale
    Gx = [al * bn_scale * gamma[i] for i in range(L)]
    Wlg = [Wl[i] * Gx[i][None, :] for i in range(L)]
    Wrg = [Wr[i] * Gx[i][None, :] for i in range(L)]
    weights = [M, Wv, Ws, Wlg[0], Wrg[0], Wlg[1], Wrg[1], Wlg[2], Wrg[2]]
    NW = len(weights)
    wpack = np.empty((P, NW * DJ * D), np.float32)
    for w, W in enumerate(weights):
        for j in range(DJ):
            wpack[:, (w * DJ + j) * D:(w * DJ + j + 1) * D] = W[j * P:(j + 1) * P, :]
    wpack = wpack.astype(ml_dtypes.bfloat16)

    Bx = [al * (bl[i] * bn_scale * gamma[i] + beta[i]) for i in range(L)]
    vecs = [bv + bs, Bx[0], Bx[0], Bx[1], Bx[1], Bx[2], Bx[2]]
    vpack = np.concatenate(vecs)[None, :].astype(ml_dtypes.bfloat16)

    in_maps = []
    for r in range(NC):
        idx_arr = np.zeros((P, SC * 8), np.int16)
        msk_arr = np.zeros((P, SC, P), NP_FP8)
        ivd_arr = np.zeros((P, nt), np.float32)
        for k in range(nt):
            tloc = int(perms[r][k])
            St = int(S_list[k])
            g = r * nt + tloc
            uniq = uniq_per_tile[g]
            nu = len(uniq)
            e0, e1 = starts[g], starts[g + 1]
            # multiplicity mask [St*P src-slot, P dst-local]
            mask = np.zeros((St * P, P), np.float32)
            if nu:
                inv = np.searchsorted(uniq, src_s[e0:e1])
                np.add.at(mask, (inv, dst_s[e0:e1] - g * P), 1.0)
            srcs = np.zeros(St * P, np.int64)
            srcs[:nu] = pos_of_node[uniq]
            o = int(offs[k])
            idx_arr[:, o * 8:(o + St) * 8] = _wrap_idx(srcs)
            # SBUF layout [128 part=src%128, chunk, dstcol]
            m3 = mask.reshape(St, P, P).transpose(1, 0, 2)
            msk_arr[:, o:o + St, :] = m3.astype(NP_FP8)
            ivd_arr[:, k] = invdeg_full[g * P:(g + 1) * P]
        pr = perms[r]

        xt_r = np.empty((P, DJ * sh), np.float32)
        for j in range(DJ):
            xs = xT[j * P:(j + 1) * P, r * sh:(r + 1) * sh]      # [P, sh]
            xs = xs.reshape(P, nt, P)[:, pr, :].reshape(P, sh)   # permute tiles
            xt_r[:, j * sh:(j + 1) * sh] = xs

        in_maps.append({
            "xt_in": xt_r.astype(ml_dtypes.bfloat16),
            "wpack_in": wpack,
            "vpack_in": vpack,
            "idx_in": idx_arr,
            "msk_in": msk_arr,
            "ivd_in": ivd_arr,
            "xtab_in": xtab,
            "xtab8_in": xtab8,
        })
    return in_maps, perms, (n_pad, sh, nt, tuple(int(s) for s in S_list), oma)


def kernel(**inputs):
    x = np.asarray(inputs["x"], np.float32)
    edge_index = np.asarray(inputs["edge_index"])
    args = dict(
        Wq=np.asarray(inputs["Wq"], np.float32), bq=np.asarray(inputs["bq"], np.float32),
        Wk=np.asarray(inputs["Wk"], np.float32), bk=np.asarray(inputs["bk"], np.float32),
        Wv=np.asarray(inputs["Wv"], np.float32), bv=np.asarray(inputs["bv"], np.float32),
        Ws=np.asarray(inputs["Ws"], np.float32), bs=np.asarray(inputs["bs"], np.float32),
        Wl=np.asarray(inputs["Wl"], np.float32), bl=np.asarray(inputs["bl"], np.float32),
        Wr=np.asarray(inputs["Wr"], np.float32),
        gamma=np.asarray(inputs["gamma"], np.float32),
        beta=np.asarray(inputs["beta"], np.float32),
        alpha_res=float(np.asarray(inputs["alpha_res"])),
    )
    src = edge_index[0].astype(np.int64)
    dst = edge_index[1].astype(np.int64)

    in_maps, perms, params = _host_prep(x, src, dst, **args)
    n_pad, sh, nt, S_list, oma = params
    t0 = time.time()
    nc = build_nc(*params)
    print(f"[kernel] build+compile {time.time()-t0:.1f}s", flush=True)
    t0 = time.time()
    res = run_bass_kernel_spmd(nc, in_maps, core_ids=list(range(NC)))
    print(f"[kernel] run {time.time()-t0:.1f}s", flush=True)
    # rows come back slot-ordered; un-permute to natural node order
    outs = []
    for r in range(NC):
        o = np.asarray(res.results[r]["out"]).astype(np.float32).reshape(nt, P, D)
        outs.append(o[np.argsort(perms[r])].reshape(sh, D))
    out = np.concatenate(outs, axis=0)
    return out[:x.shape[0]]
